# revision 59
# baseline (speedup 1.0000x reference)
"""MoD (mixture-of-depths) attention Bass kernel for Trainium2, 8 NeuronCores.

Problem: B=4, L=4096, D=1024, H=16, HD=64, K=1024 (top-25% tokens per row).
  scores = x @ w_router + b ; idx = top_k(scores, 1024) (desc order)
  xs = x[idx]; causal attention over score-ordered subsequence; out = x with
  selected rows replaced by attention output.

Sharding: core pair (2b, 2b+1) handles batch row b. Within a pair the 16
heads are split 8/8 (tensor parallel). Each core computes half the router
scores in fp32 (AllGather within pair), full top-k redundantly, gathers xs
(bf16) via indirect DMA, computes QKV for its 8 heads, causal attention, and
a partial out-projection over its 512 e-dims; a pair ReduceScatter(add) in
bf16 yields each core's 512-token half of the final attention output.
Host reassembles: out[b] = x[b].copy(); out[b][idx] = concat(halves).

All heavy matmuls run in bf16 (1 PE cycle/row vs 4 for fp32). Router
scores are computed from fp16 x with fp32 products/accumulation (verified
flip-free against the fp32 ordering for this input distribution). Tokens
are NOT fully sorted by score: they are split into two exact 512-token
groups (above/below the 512th-largest score, via one extra kth_largest
call riding on round 1's masking) which preserves the block-causal tile
structure; exact causality within the A-A and B-B tiles comes from 8
additive rank-mask tiles (-1e30 where rank[tq] < rank[tk]) accumulated
into PSUM by a bf16 identity matmul before the S matmul (B-queries vs
A-keys tiles are always fully valid and run unmasked). This keeps the
full score-sort permutation off the critical path entirely: the x gather
starts straight from the compacted indices. The host output scatter pairs
idx_out[t] with y row t, so token order is free. Softmax denominators are
broadcast across partitions with a rank-1 ones matmul instead of a DRAM
round-trip; QKV, attention and out-projection are software-pipelined per
query half with the normalize and out-projection deferred so the PE never
stalls on DVE/Act results.
"""

import numpy as np
import ml_dtypes

import concourse.bass as bass
import concourse.bacc as bacc
import concourse.mybir as mybir
import concourse.tile as tile
from concourse.bass import IndirectOffsetOnAxis
from concourse.bass_utils import run_bass_kernel_spmd

F32 = mybir.dt.float32
BF16 = mybir.dt.bfloat16
I32 = mybir.dt.int32
AF = mybir.ActivationFunctionType
OP = mybir.AluOpType
BF_NP = ml_dtypes.bfloat16

B, L, D = 4, 4096, 1024
H, HD = 16, 64
K = 1024
SCALE = 1.0 / 8.0
NEG = -1.0e30
EH = 512          # e-dims per core (8 heads)
NH_OWN = 8        # heads per core
N_TC = 8          # token chunks of 128 (K = 1024)
N_EBLK = 4        # e-blocks of 128 per core
YDT = BF16  # ReduceScatter / output dtype (host converts back to f32)


def _consts():
    """Inline constant tensors (baked into the NEFF, DMA'd at load time)."""
    c = {}
    c["identity"] = np.eye(128, dtype=np.float32)
    c["ident_bf"] = np.eye(128, dtype=BF_NP)
    c["ones64"] = np.ones((1, 64), dtype=BF_NP)
    # compaction index iota over [16, 256]: value = p*256 + f (fp32)
    c["iota16"] = (np.arange(16)[:, None] * 256 + np.arange(256)[None, :]).astype(
        np.float32
    )
    c["negones16"] = np.full((16, 256), -1.0, dtype=np.float32)
    return c


def _quantile_for(n_valid, k_adj):
    # kth_largest: k_adj = floor(omq * (n_valid - 1) / 2^32); out[1] = desc[k_adj + 1]
    return 1.0 - (k_adj + 0.5) / (n_valid - 1)


def build_program(n_cores=8, percore_shapes=False):
    """Builds the SPMD Bass program (same program on all cores; per-core
    behavior comes only from per-core input data). n_cores=1 builds the
    collective-free single-core variant (for simulation): full scores on the
    one core and no ReduceScatter (ypart is the output)."""
    spmd = n_cores > 1
    L_OWN = L // 2 if (spmd or percore_shapes) else L
    NSC = L_OWN // 128  # score tiles

    nc = bacc.Bacc("TRN2", num_devices=n_cores, debug=False)

    # ---- I/O ----
    x_bf = nc.dram_tensor("x_bf", [L, D], BF16, kind="ExternalInput")
    x_sc = nc.dram_tensor("x_sc", [L_OWN, D], mybir.dt.float16, kind="ExternalInput")
    w_rt = nc.dram_tensor("w_rt", [1, D], F32, kind="ExternalInput")
    b_rt = nc.dram_tensor("b_rt", [1, 1], F32, kind="ExternalInput")
    # wq/wk tiled host-side: [eblk, 128d(part), dblk, 128e] bf16
    wq_t = nc.dram_tensor("wq_t", [N_EBLK, 128, 8, 128], BF16, kind="ExternalInput")
    wk_t = nc.dram_tensor("wk_t", [N_EBLK, 128, 8, 128], BF16, kind="ExternalInput")
    wv_o = nc.dram_tensor("wv_o", [D, EH], BF16, kind="ExternalInput")
    wo_o = nc.dram_tensor("wo_o", [EH, D], BF16, kind="ExternalInput")

    y_out_rows = K // 2 if (spmd or percore_shapes) else K
    y_out = nc.dram_tensor("y_out", [y_out_rows, D], YDT, kind="ExternalOutput")
    idx_out = nc.dram_tensor("idx_out", [K], I32, kind="ExternalOutput")

    # ---- internal DRAM ----
    s_half_d = nc.dram_tensor("s_half_d", [L_OWN], F32, kind="Internal")
    if spmd or percore_shapes:
        s_full_d = nc.dram_tensor("s_full_d", [L], F32, kind="Internal")
        ypart_d = nc.dram_tensor("ypart_d", [K, D], YDT, kind="Internal")
        y_red_d = nc.dram_tensor("y_red_d", [K // 2, D], YDT, kind="Internal")
    else:
        s_full_d = s_half_d
    flat_v_d = nc.dram_tensor("flat_v_d", [1, K], F32, kind="Internal")
    ranks_d = nc.dram_tensor("ranks_d", [1, K], F32, kind="Internal")

    consts = {k: nc.inline_tensor(v, name=f"c_{k}") for k, v in _consts().items()}

    PAIRS = [[2 * i, 2 * i + 1] for i in range(max(n_cores // 2, 1))]

    with tile.TileContext(nc) as tc:
        with (
            tc.tile_pool(name="const", bufs=1) as cpool,
            tc.tile_pool(name="ps", bufs=5, space="PSUM") as psp,
            tc.tile_pool(name="pst", bufs=2, space="PSUM") as pstp,
            tc.tile_pool(name="psb", bufs=1, space="PSUM") as psb,
        ):
            # ---------- constants to SBUF ----------
            ident = cpool.tile([128, 128], F32)
            nc.sync.dma_start(ident[:], consts["identity"][:])
            ident_bf = cpool.tile([128, 128], BF16)
            nc.sync.dma_start(ident_bf[:], consts["ident_bf"][:])
            ones64 = cpool.tile([1, 64], BF16)
            nc.sync.dma_start(ones64[:], consts["ones64"][:])
            # rank masks (8 tiles): tokens are in grouped-compacted (not
            # score-sorted) order; exact causality "tk attends tq iff
            # rank[tq] >= rank[tk]" is applied per S^T tile. Slot m covers
            # key chunk m against its own group's query block; B-queries vs
            # A-keys tiles are always fully valid and need no mask.
            mask_sb = cpool.tile([128, 8, 512], BF16, tag="msk")

            # phase-scoped activation tensors (manual release in sequence)
            actp1 = tc.alloc_tile_pool(name="actp1", bufs=1)
            xsT = actp1.tile([128, 8, K], BF16, tag="xsT")
            i_ch_i = cpool.tile([128, 8], I32, tag="ichi")

            # attention tensors + all projection weights: allocated up front
            # so the weight DMAs stream during the score phase and V's ones
            # column is prewritten before the DVE gets busy.
            actp2 = tc.alloc_tile_pool(name="actp2", bufs=1)
            qT = actp2.tile([128, N_EBLK, K], BF16, tag="qT")
            kT = actp2.tile([128, N_EBLK, K], BF16, tag="kT")
            v_sb = actp2.tile([128, N_TC, NH_OWN, 65], BF16, tag="v")
            wall = tc.alloc_tile_pool(name="wall", bufs=1)
            wq_all = wall.tile([128, N_EBLK, 8, 128], BF16, tag="wq")
            wk_all = wall.tile([128, N_EBLK, 8, 128], BF16, tag="wk")
            wv_all = wall.tile([128, 8, 512], BF16, tag="wv")
            wo_all = wall.tile([128, N_EBLK, D], BF16, tag="wo")
            v_one = wall.tile([128, N_TC * NH_OWN], BF16, tag="vone")
            nc.vector.memset(v_one[:], 1.0)
            nc.vector.tensor_copy(
                v_sb[:, :, :, 64],
                v_one[:].rearrange("p (t h) -> p t h", t=N_TC),
            )

            # ---------- phase A: scores, top-k, gather, transposes ----------
            with (
                tc.tile_pool(name="sa", bufs=1) as spool,
                tc.tile_pool(name="sca", bufs=4) as scpool,
            ):
                iota16 = spool.tile([16, 256], F32)
                nc.sync.dma_start(iota16[:], consts["iota16"][:])
                neg16 = spool.tile([16, 256], F32)
                nc.sync.dma_start(neg16[:], consts["negones16"][:])
                w_rep = spool.tile([128, D], F32)
                nc.sync.dma_start(w_rep[:], w_rt[:].to_broadcast((128, D)))
                b_bc = spool.tile([128, 1], F32)
                nc.sync.dma_start(b_bc[:], b_rt[:].to_broadcast((128, 1)))

                # scores (own half): tile j holds x rows {p*NSC + j} so the
                # score vector lands p-major => contiguous DRAM store.
                s_half = spool.tile([128, NSC], F32)
                x_sc_v = x_sc[:].rearrange("(p j) d -> j p d", j=NSC)
                for j in range(NSC):
                    xt = scpool.tile([128, D], mybir.dt.float16, tag="x16")
                    nc.sync.dma_start(xt[:], x_sc_v[j])
                    prod = scpool.tile([128, D], F32, tag="prod")
                    nc.vector.tensor_tensor(
                        out=prod[:], in0=xt[:], in1=w_rep[:], op=OP.mult
                    )
                    acc_scr = scpool.tile([128, D], F32, tag="accscr")
                    nc.scalar.activation(
                        acc_scr[:], prod[:], AF.Copy,
                        accum_out=s_half[:, j : j + 1],
                    )
                nc.vector.tensor_scalar(
                    s_half[:], s_half[:], b_bc[:], None, op0=OP.add
                )
                nc.sync.dma_start(
                    s_half_d[:].rearrange("(p j) -> p j", j=NSC), s_half[:]
                )

                # all-gather scores within pair
                if percore_shapes:
                    nc.sync.dma_start(s_full_d[0:L_OWN], s_half_d[:])
                    nc.sync.dma_start(s_full_d[L_OWN:L], s_half_d[:])
                if spmd:
                    nc.gpsimd.collective_compute(
                        "AllGather",
                        OP.bypass,
                        replica_groups=PAIRS,
                        ins=[s_half_d[:]],
                        outs=[s_full_d[:]],
                    )

                s_sb = spool.tile([128, 32], F32)
                nc.sync.dma_start(
                    s_sb[:], s_full_d[:].rearrange("(p f) -> p f", f=32)
                )
                s16 = spool.tile([16, 256], F32)
                nc.sync.dma_start(
                    s16[:], s_full_d[:].rearrange("(p f) -> p f", f=256)
                )

                # stream all projection weights now (SP queue, behind the
                # score loads): ready well before QKV needs them
                for eblk in range(N_EBLK):
                    nc.sync.dma_start(wq_all[:, eblk], wq_t[eblk])
                    nc.sync.dma_start(wk_all[:, eblk], wk_t[eblk])
                nc.sync.dma_start(
                    wv_all[:], wv_o[:].rearrange("(k p) e -> p k e", p=128)
                )
                nc.sync.dma_start(
                    wo_all[:], wo_o[:].rearrange("(k p) d -> p k d", p=128)
                )

                # exact thresholds via masked kth rounds:
                #   T    (1024th largest) = selection threshold
                #   T512 (512th largest)  = group A/B split for causal tiling
                # Tokens land in chunks 0-3 (group A, ranks 0..511) and 4-7
                # (group B, ranks 512..1023) in compaction scan order; exact
                # intra/inter-group causality comes from the rank masks.
                s_work = spool.tile([128, 32], F32)
                nc.vector.tensor_copy(s_work[:], s_sb[:])
                negtile = spool.tile([128, 32], F32)
                nc.vector.memset(negtile[:], NEG)
                kth = spool.tile([1, 2], F32)
                t_bc = spool.tile([128, 1], F32)
                kth512 = spool.tile([1, 2], F32)
                t512_bc = spool.tile([128, 1], F32)
                def _round(n_valid, k_adj, mask_after):
                    nc.gpsimd.kth_largest(
                        kth[:], s_work[:], 32, 510,
                        quantile=_quantile_for(n_valid, k_adj),
                    )
                    nc.gpsimd.partition_broadcast(t_bc[:], kth[0:1, 1:2])
                    if mask_after:
                        ge = spool.tile([128, 32], mybir.dt.uint8, tag="gemask")
                        nc.vector.tensor_scalar(
                            ge[:], s_work[:], t_bc[:], None, op0=OP.is_ge
                        )
                        nc.vector.copy_predicated(s_work[:], ge[:], negtile[:])

                _round(4096, 508, True)
                # T512 rides on round 1's masking: the 512th largest overall
                # is the 2nd largest of the 3586 remaining
                nc.gpsimd.kth_largest(
                    kth512[:], s_work[:], 32, 510,
                    quantile=_quantile_for(3586, 0),
                )
                nc.gpsimd.partition_broadcast(t512_bc[:], kth512[0:1, 1:2])

                # group A compaction (top 512): feeds gather chunks 0-3 early
                geA = spool.tile([16, 256], mybir.dt.uint8)
                nc.vector.tensor_scalar(
                    geA[:], s16[:], t512_bc[0:16, :], None, op0=OP.is_ge
                )
                iA_in = spool.tile([16, 256], F32)
                nc.vector.tensor_copy(iA_in[:], neg16[:])
                nc.vector.copy_predicated(iA_in[:], geA[:], iota16[:])
                nfA = spool.tile([1, 1], mybir.dt.uint32)
                iA_c = spool.tile([16, 32], F32)
                nc.gpsimd.sparse_gather(iA_c[:], iA_in[:], num_found=nfA[:])

                def _to_chunks(comp, ch_cols, tag, flat_half=None):
                    """[16, 32] compacted scan order -> [4, 128] -> [128, 4]"""
                    t_ps = psb.tile([32, 16], F32, tag="tsm")
                    nc.tensor.transpose(t_ps[:], comp[:], ident[0:16, 0:16])
                    t_sb = spool.tile([32, 16], F32, tag=f"t32_{tag}")
                    nc.vector.tensor_copy(t_sb[:], t_ps[:])
                    s4 = spool.tile([4, 128], F32, tag=f"s4_{tag}")
                    nc.sync.dma_start(s4[:], t_sb[:])
                    if flat_half is not None:
                        nc.sync.dma_start(flat_half, s4[:])
                    c_ps = psb.tile([128, 4], F32, tag="tsm")
                    nc.tensor.transpose(c_ps[:], s4[:], ident[0:4, 0:4])
                    nc.vector.tensor_copy(ch_cols, c_ps[:])

                i_ch = spool.tile([128, 8], F32)
                _to_chunks(iA_c, i_ch[:, 0:4], "ia")
                nc.vector.tensor_copy(i_ch_i[:, 0:4], i_ch[:, 0:4])

                # gather group A token chunks immediately
                xs = spool.tile([128, N_TC, D], BF16, tag="xs")
                for c in range(4):
                    nc.gpsimd.indirect_dma_start(
                        out=xs[:, c, :],
                        out_offset=None,
                        in_=x_bf[:],
                        in_offset=IndirectOffsetOnAxis(
                            ap=i_ch_i[:, c : c + 1], axis=0
                        ),
                    )

                # transpose group A chunks while the remaining threshold
                # rounds run (PE + copies start ~5us earlier)
                for dblk in range(8):
                    tp = pstp.tile([128, 512], BF16, tag="ps128")
                    for cc in range(4):
                        nc.tensor.transpose(
                            tp[:, cc * 128 : (cc + 1) * 128],
                            xs[:, cc, dblk * 128 : (dblk + 1) * 128],
                            ident_bf[:],
                        )
                    nc.any.tensor_copy(xsT[:, dblk, 0:512], tp[:])

                # remaining rounds for the exact selection threshold T
                _round(3586, 508, True)
                _round(3076, 2, False)

                # group B compaction (next 512): T <= score < T512
                shifted = spool.tile([16, 256], F32)
                nc.vector.tensor_scalar(
                    shifted[:], s16[:], t_bc[0:16, :], None, op0=OP.subtract
                )
                ltB = spool.tile([16, 256], mybir.dt.uint8)
                nc.vector.tensor_scalar(
                    ltB[:], s16[:], t512_bc[0:16, :], None, op0=OP.is_lt
                )
                mask16 = spool.tile([16, 256], mybir.dt.uint8)
                nc.vector.tensor_scalar(
                    mask16[:], shifted[:], 0.0, None, op0=OP.is_ge
                )
                idx16 = spool.tile([16, 256], F32)
                nc.vector.tensor_copy(idx16[:], neg16[:])
                nc.vector.copy_predicated(idx16[:], mask16[:], iota16[:])
                iB_in = spool.tile([16, 256], F32)
                nc.vector.tensor_copy(iB_in[:], neg16[:])
                nc.vector.copy_predicated(iB_in[:], ltB[:], idx16[:])
                nfB = spool.tile([1, 1], mybir.dt.uint32)
                iB_c = spool.tile([16, 32], F32)
                nc.gpsimd.sparse_gather(iB_c[:], iB_in[:], num_found=nfB[:])
                _to_chunks(iB_c, i_ch[:, 4:8], "ib")
                nc.vector.tensor_copy(i_ch_i[:, 4:8], i_ch[:, 4:8])
                for c in range(4, N_TC):
                    nc.gpsimd.indirect_dma_start(
                        out=xs[:, c, :],
                        out_offset=None,
                        in_=x_bf[:],
                        in_offset=IndirectOffsetOnAxis(
                            ap=i_ch_i[:, c : c + 1], axis=0
                        ),
                    )
                # idx_out [1024]: token t = c*128 + p is the t-th selected
                # index (host pairs idx_out[t] with y row t; order is free)
                nc.sync.dma_start(
                    idx_out[:].rearrange("(c p) -> p c", p=128), i_ch_i[:]
                )

                # transpose group B chunks
                for dblk in range(8):
                    tp = pstp.tile([128, 512], BF16, tag="ps128")
                    for cc in range(4):
                        nc.tensor.transpose(
                            tp[:, cc * 128 : (cc + 1) * 128],
                            xs[:, 4 + cc, dblk * 128 : (dblk + 1) * 128],
                            ident_bf[:],
                        )
                    nc.any.tensor_copy(xsT[:, dblk, 512:1024], tp[:])

                # --- value side: shifted scores -> global desc ranks ->
                # additive rank masks (off the gather critical path) ---
                vA_in = spool.tile([16, 256], F32)
                nc.vector.tensor_copy(vA_in[:], neg16[:])
                nc.vector.copy_predicated(vA_in[:], geA[:], shifted[:])
                vB_in = spool.tile([16, 256], F32)
                nc.vector.tensor_copy(vB_in[:], neg16[:])
                nc.vector.copy_predicated(vB_in[:], ltB[:], shifted[:])
                nfVA = spool.tile([1, 1], mybir.dt.uint32)
                vA_c = spool.tile([16, 32], F32)
                nc.gpsimd.sparse_gather(vA_c[:], vA_in[:], num_found=nfVA[:])
                nfVB = spool.tile([1, 1], mybir.dt.uint32)
                vB_c = spool.tile([16, 32], F32)
                nc.gpsimd.sparse_gather(vB_c[:], vB_in[:], num_found=nfVB[:])
                v_ch = spool.tile([128, 8], F32)
                _to_chunks(vA_c, v_ch[:, 0:4], "va", flat_v_d[:, 0:512])
                _to_chunks(vB_c, v_ch[:, 4:8], "vb", flat_v_d[:, 512:K])
                rep = spool.tile([128, K], F32)
                nc.gpsimd.dma_start(rep[:], flat_v_d[:].to_broadcast((128, K)))

                # rank[p, c] = #selected values greater (desc rank, 0-based)
                ranks = spool.tile([128, 8], F32)
                for c in range(8):
                    rankscr = scpool.tile([128, K], F32, tag="xsc")
                    nc.vector.tensor_tensor(
                        out=rankscr[:], in0=rep[:],
                        in1=v_ch[:, c : c + 1].to_broadcast((128, K)),
                        op=OP.is_gt,
                    )
                    rankscr2 = scpool.tile([128, K], F32, tag="prod")
                    nc.scalar.activation(
                        rankscr2[:], rankscr[:], AF.Copy,
                        accum_out=ranks[:, c : c + 1],
                    )
                # flat rank vector in token order, broadcast to all partitions
                # (Pool DMA queue: keeps the SP queue free for weight loads)
                nc.gpsimd.dma_start(
                    ranks_d[0].rearrange("(c p) -> p c", p=128), ranks[:]
                )
                rank_rep = spool.tile([128, K], F32)
                nc.gpsimd.dma_start(
                    rank_rep[:], ranks_d[:].to_broadcast((128, K))
                )
                # mask slot m: key chunk m vs its own group's query block;
                # -1e30 where rank[tq] < rank[tk]
                for m in range(8):
                    n = m // 4
                    nc.vector.tensor_scalar(
                        mask_sb[:, m, :],
                        rank_rep[:, n * 512 : (n + 1) * 512],
                        ranks[:, m : m + 1], NEG,
                        op0=OP.is_lt, op1=OP.mult,
                    )

            # ---------- fused QKV + attention, one query half at a time ----
            # QKV for token half n, then attention for query block n (which
            # only needs kT/v up to half n and qT of half n), then that
            # half's out-projection; the next half's QKV matmuls overlap the
            # exp/normalize drain of this half. Within the m loop, S(m+1) is
            # issued before PV(m) so PE stays ahead of the exp dependency.
            actp3 = tc.alloc_tile_pool(name="actp3", bufs=1)
            oT = actp3.tile([128, N_EBLK, K], BF16, tag="oT")
            ydst = ypart_d if (spmd or percore_shapes) else y_out
            expp = tc.alloc_tile_pool(name="expp", bufs=6)

            # out-projection for query half nn: emitted one half late so the
            # next half's QKV matmuls hide the wait on oT completion
            def _outproj(nn):
                for tci in range(4):
                    tc_i = nn * 4 + tci
                    for dc in range(2):
                        py = psp.tile([128, 512], F32, tag="ps512")
                        for eblk in range(N_EBLK):
                            nc.tensor.matmul(
                                py[:],
                                oT[:, eblk, tc_i * 128 : (tc_i + 1) * 128],
                                wo_all[:, eblk, dc * 512 : (dc + 1) * 512],
                                start=(eblk == 0), stop=(eblk == N_EBLK - 1),
                            )
                        y_sb = expp.tile([128, 512], YDT, tag="ysb")
                        nc.vector.tensor_copy(y_sb[:], py[:])
                        # percore sim: own token half goes straight to y_out
                        # (spmd instead ReduceScatters the full ypart)
                        if percore_shapes and not spmd and tc_i < 4:
                            dst = y_out
                        else:
                            dst = ydst
                        nc.sync.dma_start(
                            dst[tc_i * 128 : (tc_i + 1) * 128,
                                dc * 512 : (dc + 1) * 512],
                            y_sb[:],
                        )

            for n in range(2):
                tql = bass.ts(n, 512)
                n_m = 4 * n + 4
                # QKV for this token half
                for eblk in range(N_EBLK):
                    pq = psp.tile([128, 512], F32, tag="ps512")
                    pk = psp.tile([128, 512], F32, tag="ps512")
                    for dblk in range(8):
                        nc.tensor.matmul(
                            pq[:], wq_all[:, eblk, dblk, :], xsT[:, dblk, tql],
                            start=(dblk == 0), stop=(dblk == 7),
                        )
                    for dblk in range(8):
                        nc.tensor.matmul(
                            pk[:], wk_all[:, eblk, dblk, :], xsT[:, dblk, tql],
                            start=(dblk == 0), stop=(dblk == 7),
                        )
                    nc.any.tensor_copy(qT[:, eblk, tql], pq[:])
                    nc.any.tensor_copy(kT[:, eblk, tql], pk[:])
                for tc_i in range(4 * n, 4 * n + 4):
                    pv = psp.tile([128, 512], F32, tag="ps512")
                    for dblk in range(8):
                        nc.tensor.matmul(
                            pv[:],
                            xsT[:, dblk, tc_i * 128 : (tc_i + 1) * 128],
                            wv_all[:, dblk, :],
                            start=(dblk == 0), stop=(dblk == 7),
                        )
                    nc.any.tensor_copy(
                        v_sb[:, tc_i, :, 0:64],
                        pv[:].rearrange("p (h e) -> p h e", h=8),
                    )
                if n == 1:
                    _outproj(0)

                # normalize rows 0..63 of po by row 64 (ones-matmul bcast);
                # deferred one head so the PE never waits on the reciprocal
                def _normalize(po, esl, eblk):
                    r_row = expp.tile([1, 512], BF16, tag="rrow")
                    with nc.allow_low_precision(reason="softmax recip bf16"):
                        nc.vector.reciprocal(r_row[:], po[64:65, :])
                    r_bc = pstp.tile([64, 512], F32, tag="ps128")
                    nc.tensor.matmul(
                        r_bc[:], ones64[:], r_row[:], start=True, stop=True
                    )
                    r_sb = expp.tile([64, 512], BF16, tag="rsb")
                    nc.vector.tensor_copy(r_sb[:], r_bc[:])
                    nc.vector.tensor_tensor(
                        out=oT[esl, eblk, tql],
                        in0=po[0:64, :], in1=r_sb[:], op=OP.mult,
                    )

                pending_norm = None
                for eblk in range(N_EBLK):
                    for sub in range(2):
                        hh = eblk * 2 + sub
                        esl = slice(sub * 64, sub * 64 + 64)
                        po = psp.tile([65, 512], F32, tag="ps512")
                        prev_es = None
                        for m in range(n_m):
                            ps_s = psp.tile([128, 512], F32, tag="ps512")
                            # B-queries vs A-keys (n=1, m<4) are fully valid
                            need_mask = (n == 0) or (m >= 4)
                            if need_mask:
                                nc.tensor.matmul(
                                    ps_s[:], ident_bf[:],
                                    mask_sb[:, m, :],
                                    start=True, stop=False,
                                )
                            nc.tensor.matmul(
                                ps_s[:],
                                kT[esl, eblk, m * 128 : (m + 1) * 128],
                                qT[esl, eblk, tql],
                                start=not need_mask, stop=True,
                                tile_position=(sub * 64, 0),
                            )
                            es = expp.tile([128, 512], BF16, tag="es")
                            nc.scalar.activation(
                                es[:], ps_s[:], AF.Exp, scale=SCALE
                            )
                            if prev_es is not None:
                                nc.tensor.matmul(
                                    po[:], v_sb[:, m - 1, hh, :], prev_es[:],
                                    start=(m == 1), stop=False,
                                )
                            prev_es = es
                        nc.tensor.matmul(
                            po[:], v_sb[:, n_m - 1, hh, :], prev_es[:],
                            start=(n_m == 1), stop=True,
                        )
                        if pending_norm is not None:
                            _normalize(*pending_norm)
                        pending_norm = (po, esl, eblk)
                _normalize(*pending_norm)

            _outproj(1)

            expp.release()
            actp3.release()
            wall.release()
            actp2.release()
            actp1.release()

            if spmd:
                nc.gpsimd.collective_compute(
                    "ReduceScatter",
                    OP.add,
                    replica_groups=PAIRS,
                    ins=[ypart_d[:]],
                    outs=[y_red_d[:]],
                )
                nc.sync.dma_start(y_out[:], y_red_d[:])

    nc.compile()
    return nc


_NC_CACHE = {}


def _get_nc(n_cores=8):
    if n_cores not in _NC_CACHE:
        _NC_CACHE[n_cores] = build_program(n_cores)
    return _NC_CACHE[n_cores]


def _weight_tiles(w_half):
    # [1024, 512] -> [eblk, 128d(part), dblk, 128e]; inner 8*128 contiguous
    # per partition row so the SBUF load streams 2KB descriptors.
    return np.ascontiguousarray(
        w_half.reshape(8, 128, 4, 128).transpose(2, 1, 0, 3).astype(BF_NP)
    )


def _build_in_maps(inputs):
    x = np.ascontiguousarray(np.asarray(inputs["x"], np.float32))
    w_router = np.asarray(inputs["w_router"], np.float32)
    b_router = np.asarray(inputs["b_router"], np.float32)
    wq = np.asarray(inputs["wq"], np.float32)
    wk = np.asarray(inputs["wk"], np.float32)
    wv = np.asarray(inputs["wv"], np.float32)
    wo = np.asarray(inputs["wo"], np.float32)
    x_bf = x.astype(BF_NP)

    in_maps = []
    for core in range(8):
        b = core // 2
        half = core % 2
        esl = slice(half * EH, (half + 1) * EH)
        in_maps.append(
            {
                "x_bf": x_bf[b],
                "x_sc": np.ascontiguousarray(
                    x[b, half * 2048 : (half + 1) * 2048]
                ).astype(np.float16),
                "w_rt": w_router.reshape(1, D),
                "b_rt": b_router.reshape(1, 1),
                "wq_t": _weight_tiles(wq[:, esl]),
                "wk_t": _weight_tiles(wk[:, esl]),
                "wv_o": np.ascontiguousarray(wv[:, esl]).astype(BF_NP),
                "wo_o": np.ascontiguousarray(wo[esl, :]).astype(BF_NP),
            }
        )
    return in_maps


def kernel(x, w_router, b_router, wq, wk, wv, wo):
    x = np.asarray(x, np.float32)
    nc = _get_nc(8)
    in_maps = _build_in_maps(
        dict(x=x, w_router=w_router, b_router=b_router, wq=wq, wk=wk, wv=wv, wo=wo)
    )
    res = run_bass_kernel_spmd(nc, in_maps, core_ids=list(range(8)))
    out = x.copy()
    for b in range(B):
        idx = res.results[2 * b]["idx_out"].astype(np.int64)
        y = np.concatenate(
            [
                np.asarray(res.results[2 * b]["y_out"], np.float32),
                np.asarray(res.results[2 * b + 1]["y_out"], np.float32),
            ],
            axis=0,
        )
        out[b][idx] = y
    return out


# revision 69
# speedup vs baseline: 1.0013x; 1.0013x over previous
"""MoD (mixture-of-depths) attention Bass kernel for Trainium2, 8 NeuronCores.

Problem: B=4, L=4096, D=1024, H=16, HD=64, K=1024 (top-25% tokens per row).
  scores = x @ w_router + b ; idx = top_k(scores, 1024) (desc order)
  xs = x[idx]; causal attention over score-ordered subsequence; out = x with
  selected rows replaced by attention output.

Sharding: core pair (2b, 2b+1) handles batch row b. Within a pair the 16
heads are split 8/8 (tensor parallel). Each core computes half the router
scores in fp32 (AllGather within pair), full top-k redundantly, gathers xs
(bf16) via indirect DMA, computes QKV for its 8 heads, causal attention, and
a partial out-projection over its 512 e-dims; a pair ReduceScatter(add) in
bf16 yields each core's 512-token half of the final attention output.
Host reassembles: out[b] = x[b].copy(); out[b][idx] = concat(halves).

All heavy matmuls run in bf16 (1 PE cycle/row vs 4 for fp32). Router
scores are computed from fp16 x with fp32 products/accumulation (verified
flip-free against the fp32 ordering for this input distribution). Tokens
are NOT fully sorted by score: they are split into two exact 512-token
groups (above/below the 512th-largest score, via one extra kth_largest
call riding on round 1's masking) which preserves the block-causal tile
structure; exact causality within the A-A and B-B tiles comes from 8
additive rank-mask tiles (-1e30 where rank[tq] < rank[tk]) accumulated
into PSUM by a bf16 identity matmul before the S matmul (B-queries vs
A-keys tiles are always fully valid and run unmasked). This keeps the
full score-sort permutation off the critical path entirely: the x gather
starts straight from the compacted indices. The host output scatter pairs
idx_out[t] with y row t, so token order is free. Softmax denominators are
broadcast across partitions with a rank-1 ones matmul instead of a DRAM
round-trip; QKV, attention and out-projection are software-pipelined per
query half with the normalize and out-projection deferred so the PE never
stalls on DVE/Act results.
"""

import numpy as np
import ml_dtypes

import concourse.bass as bass
import concourse.bacc as bacc
import concourse.mybir as mybir
import concourse.tile as tile
from concourse.bass import IndirectOffsetOnAxis
from concourse.bass_utils import run_bass_kernel_spmd

F32 = mybir.dt.float32
BF16 = mybir.dt.bfloat16
I32 = mybir.dt.int32
AF = mybir.ActivationFunctionType
OP = mybir.AluOpType
BF_NP = ml_dtypes.bfloat16

B, L, D = 4, 4096, 1024
H, HD = 16, 64
K = 1024
SCALE = 1.0 / 8.0
NEG = -1.0e30
EH = 512          # e-dims per core (8 heads)
NH_OWN = 8        # heads per core
N_TC = 8          # token chunks of 128 (K = 1024)
N_EBLK = 4        # e-blocks of 128 per core
YDT = BF16  # ReduceScatter / output dtype (host converts back to f32)


def _consts():
    """Inline constant tensors (baked into the NEFF, DMA'd at load time)."""
    c = {}
    c["identity"] = np.eye(128, dtype=np.float32)
    c["ident_bf"] = np.eye(128, dtype=BF_NP)
    c["ones64"] = np.ones((1, 64), dtype=BF_NP)
    # compaction index iota over [16, 256]: value = p*256 + f (fp32)
    c["iota16"] = (np.arange(16)[:, None] * 256 + np.arange(256)[None, :]).astype(
        np.float32
    )
    c["negones16"] = np.full((16, 256), -1.0, dtype=np.float32)
    return c


def _quantile_for(n_valid, k_adj):
    # kth_largest: k_adj = floor(omq * (n_valid - 1) / 2^32); out[1] = desc[k_adj + 1]
    return 1.0 - (k_adj + 0.5) / (n_valid - 1)


def build_program(n_cores=8, percore_shapes=False):
    """Builds the SPMD Bass program (same program on all cores; per-core
    behavior comes only from per-core input data). n_cores=1 builds the
    collective-free single-core variant (for simulation): full scores on the
    one core and no ReduceScatter (ypart is the output)."""
    spmd = n_cores > 1
    L_OWN = L // 2 if (spmd or percore_shapes) else L
    NSC = L_OWN // 128  # score tiles

    nc = bacc.Bacc("TRN2", num_devices=n_cores, debug=False)

    # ---- I/O ----
    x_bf = nc.dram_tensor("x_bf", [L, D], BF16, kind="ExternalInput")
    x_sc = nc.dram_tensor("x_sc", [L_OWN, D], mybir.dt.float16, kind="ExternalInput")
    w_rt = nc.dram_tensor("w_rt", [1, D], F32, kind="ExternalInput")
    b_rt = nc.dram_tensor("b_rt", [1, 1], F32, kind="ExternalInput")
    # wq/wk tiled host-side: [eblk, 128d(part), dblk, 128e] bf16
    wq_t = nc.dram_tensor("wq_t", [N_EBLK, 128, 8, 128], BF16, kind="ExternalInput")
    wk_t = nc.dram_tensor("wk_t", [N_EBLK, 128, 8, 128], BF16, kind="ExternalInput")
    wv_o = nc.dram_tensor("wv_o", [D, EH], BF16, kind="ExternalInput")
    wo_o = nc.dram_tensor("wo_o", [EH, D], BF16, kind="ExternalInput")

    y_out_rows = K // 2 if (spmd or percore_shapes) else K
    y_out = nc.dram_tensor("y_out", [y_out_rows, D], YDT, kind="ExternalOutput")
    idx_out = nc.dram_tensor("idx_out", [K], I32, kind="ExternalOutput")

    # ---- internal DRAM ----
    s_half_d = nc.dram_tensor("s_half_d", [L_OWN], F32, kind="Internal")
    if spmd or percore_shapes:
        s_full_d = nc.dram_tensor("s_full_d", [L], F32, kind="Internal")
        ypart_d = nc.dram_tensor("ypart_d", [K, D], YDT, kind="Internal")
        y_red_d = nc.dram_tensor("y_red_d", [K // 2, D], YDT, kind="Internal")
    else:
        s_full_d = s_half_d
    flat_v_d = nc.dram_tensor("flat_v_d", [1, K], F32, kind="Internal")
    ranks_d = nc.dram_tensor("ranks_d", [1, K], F32, kind="Internal")

    consts = {k: nc.inline_tensor(v, name=f"c_{k}") for k, v in _consts().items()}

    PAIRS = [[2 * i, 2 * i + 1] for i in range(max(n_cores // 2, 1))]

    with tile.TileContext(nc) as tc:
        with (
            tc.tile_pool(name="const", bufs=1) as cpool,
            tc.tile_pool(name="ps", bufs=5, space="PSUM") as psp,
            tc.tile_pool(name="pst", bufs=2, space="PSUM") as pstp,
            tc.tile_pool(name="psb", bufs=1, space="PSUM") as psb,
        ):
            # ---------- constants to SBUF ----------
            ident = cpool.tile([128, 128], F32)
            nc.sync.dma_start(ident[:], consts["identity"][:])
            ident_bf = cpool.tile([128, 128], BF16)
            nc.sync.dma_start(ident_bf[:], consts["ident_bf"][:])
            ones64 = cpool.tile([1, 64], BF16)
            nc.sync.dma_start(ones64[:], consts["ones64"][:])
            # rank masks (8 tiles): tokens are in grouped-compacted (not
            # score-sorted) order; exact causality "tk attends tq iff
            # rank[tq] >= rank[tk]" is applied per S^T tile. Slot m covers
            # key chunk m against its own group's query block; B-queries vs
            # A-keys tiles are always fully valid and need no mask.
            mask_sb = cpool.tile([128, 8, 512], BF16, tag="msk")

            # phase-scoped activation tensors (manual release in sequence)
            actp1 = tc.alloc_tile_pool(name="actp1", bufs=1)
            xsT = actp1.tile([128, 8, K], BF16, tag="xsT")
            i_ch_i = cpool.tile([128, 8], I32, tag="ichi")

            # attention tensors + all projection weights: allocated up front
            # so the weight DMAs stream during the score phase and V's ones
            # column is prewritten before the DVE gets busy.
            actp2 = tc.alloc_tile_pool(name="actp2", bufs=1)
            qT = actp2.tile([128, N_EBLK, K], BF16, tag="qT")
            kT = actp2.tile([128, N_EBLK, K], BF16, tag="kT")
            v_sb = actp2.tile([128, N_TC, NH_OWN, 65], BF16, tag="v")
            wall = tc.alloc_tile_pool(name="wall", bufs=1)
            wq_all = wall.tile([128, N_EBLK, 8, 128], BF16, tag="wq")
            wk_all = wall.tile([128, N_EBLK, 8, 128], BF16, tag="wk")
            wv_all = wall.tile([128, 8, 512], BF16, tag="wv")
            wo_all = wall.tile([128, N_EBLK, D], BF16, tag="wo")
            v_one = wall.tile([128, N_TC * NH_OWN], BF16, tag="vone")
            nc.vector.memset(v_one[:], 1.0)
            nc.vector.tensor_copy(
                v_sb[:, :, :, 64],
                v_one[:].rearrange("p (t h) -> p t h", t=N_TC),
            )

            # ---------- phase A: scores, top-k, gather, transposes ----------
            with (
                tc.tile_pool(name="sa", bufs=1) as spool,
                tc.tile_pool(name="sca", bufs=4) as scpool,
            ):
                iota16 = spool.tile([16, 256], F32)
                nc.sync.dma_start(iota16[:], consts["iota16"][:])
                neg16 = spool.tile([16, 256], F32)
                nc.sync.dma_start(neg16[:], consts["negones16"][:])
                w_rep = spool.tile([128, D], F32)
                nc.sync.dma_start(w_rep[:], w_rt[:].to_broadcast((128, D)))
                b_bc = spool.tile([128, 1], F32)
                nc.sync.dma_start(b_bc[:], b_rt[:].to_broadcast((128, 1)))

                # scores (own half): tile j holds x rows {p*NSC + j} so the
                # score vector lands p-major => contiguous DRAM store.
                s_half = spool.tile([128, NSC], F32)
                x_sc_v = x_sc[:].rearrange("(p j) d -> j p d", j=NSC)
                for j in range(NSC):
                    xt = scpool.tile([128, D], mybir.dt.float16, tag="x16")
                    nc.sync.dma_start(xt[:], x_sc_v[j])
                    prod = scpool.tile([128, D], F32, tag="prod")
                    nc.vector.tensor_tensor(
                        out=prod[:], in0=xt[:], in1=w_rep[:], op=OP.mult
                    )
                    acc_scr = scpool.tile([128, D], F32, tag="accscr")
                    nc.scalar.activation(
                        acc_scr[:], prod[:], AF.Copy,
                        accum_out=s_half[:, j : j + 1],
                    )
                nc.vector.tensor_scalar(
                    s_half[:], s_half[:], b_bc[:], None, op0=OP.add
                )
                nc.sync.dma_start(
                    s_half_d[:].rearrange("(p j) -> p j", j=NSC), s_half[:]
                )

                # all-gather scores within pair
                if percore_shapes:
                    nc.sync.dma_start(s_full_d[0:L_OWN], s_half_d[:])
                    nc.sync.dma_start(s_full_d[L_OWN:L], s_half_d[:])
                if spmd:
                    nc.gpsimd.collective_compute(
                        "AllGather",
                        OP.bypass,
                        replica_groups=PAIRS,
                        ins=[s_half_d[:]],
                        outs=[s_full_d[:]],
                    )

                s_sb = spool.tile([128, 32], F32)
                nc.sync.dma_start(
                    s_sb[:], s_full_d[:].rearrange("(p f) -> p f", f=32)
                )
                s16 = spool.tile([16, 256], F32)
                nc.sync.dma_start(
                    s16[:], s_full_d[:].rearrange("(p f) -> p f", f=256)
                )

                # stream all projection weights now (SP queue, behind the
                # score loads): ready well before QKV needs them
                for eblk in range(N_EBLK):
                    nc.sync.dma_start(wq_all[:, eblk], wq_t[eblk])
                    nc.sync.dma_start(wk_all[:, eblk], wk_t[eblk])
                nc.sync.dma_start(
                    wv_all[:], wv_o[:].rearrange("(k p) e -> p k e", p=128)
                )
                nc.sync.dma_start(
                    wo_all[:], wo_o[:].rearrange("(k p) d -> p k d", p=128)
                )

                # exact thresholds via masked kth rounds:
                #   T    (1024th largest) = selection threshold
                #   T512 (512th largest)  = group A/B split for causal tiling
                # Tokens land in chunks 0-3 (group A, ranks 0..511) and 4-7
                # (group B, ranks 512..1023) in compaction scan order; exact
                # intra/inter-group causality comes from the rank masks.
                # threshold rounds mask s_sb in place (s16 keeps the
                # original values for the compaction step)
                s_work = s_sb
                negtile = spool.tile([128, 32], F32)
                nc.vector.memset(negtile[:], NEG)
                kth = spool.tile([1, 2], F32)
                t_bc = spool.tile([128, 1], F32)
                kth512 = spool.tile([1, 2], F32)
                t512_bc = spool.tile([128, 1], F32)
                # pre-stage the -1 sentinel tiles for the compaction
                # selects while the score AllGather lands (off the
                # post-threshold critical chain)
                iA_in = spool.tile([16, 256], F32)
                nc.vector.tensor_copy(iA_in[:], neg16[:])
                idx16 = spool.tile([16, 256], F32)
                nc.vector.tensor_copy(idx16[:], neg16[:])
                iB_in = spool.tile([16, 256], F32)
                nc.vector.tensor_copy(iB_in[:], neg16[:])
                vA_in = spool.tile([16, 256], F32)
                nc.vector.tensor_copy(vA_in[:], neg16[:])
                vB_in = spool.tile([16, 256], F32)
                nc.vector.tensor_copy(vB_in[:], neg16[:])

                def _round(n_valid, k_adj, mask_after):
                    nc.gpsimd.kth_largest(
                        kth[:], s_work[:], 32, 510,
                        quantile=_quantile_for(n_valid, k_adj),
                    )
                    nc.gpsimd.partition_broadcast(t_bc[:], kth[0:1, 1:2])
                    if mask_after:
                        ge = spool.tile([128, 32], mybir.dt.uint8, tag="gemask")
                        nc.vector.tensor_scalar(
                            ge[:], s_work[:], t_bc[:], None, op0=OP.is_ge
                        )
                        nc.vector.copy_predicated(s_work[:], ge[:], negtile[:])

                _round(4096, 508, True)
                # T512 rides on round 1's masking: the 512th largest overall
                # is the 2nd largest of the 3586 remaining
                nc.gpsimd.kth_largest(
                    kth512[:], s_work[:], 32, 510,
                    quantile=_quantile_for(3586, 0),
                )
                nc.gpsimd.partition_broadcast(t512_bc[:], kth512[0:1, 1:2])

                # group A compaction (top 512): feeds gather chunks 0-3 early
                geA = spool.tile([16, 256], mybir.dt.uint8)
                nc.vector.tensor_scalar(
                    geA[:], s16[:], t512_bc[0:16, :], None, op0=OP.is_ge
                )
                nc.vector.copy_predicated(iA_in[:], geA[:], iota16[:])
                nfA = spool.tile([1, 1], mybir.dt.uint32)
                iA_c = spool.tile([16, 32], F32)
                nc.gpsimd.sparse_gather(iA_c[:], iA_in[:], num_found=nfA[:])

                def _to_chunks(comp, ch_cols, tag, flat_half=None):
                    """[16, 32] compacted scan order -> [4, 128] -> [128, 4]"""
                    t_ps = psb.tile([32, 16], F32, tag="tsm")
                    nc.tensor.transpose(t_ps[:], comp[:], ident[0:16, 0:16])
                    t_sb = spool.tile([32, 16], F32, tag=f"t32_{tag}")
                    nc.vector.tensor_copy(t_sb[:], t_ps[:])
                    s4 = spool.tile([4, 128], F32, tag=f"s4_{tag}")
                    nc.sync.dma_start(s4[:], t_sb[:])
                    if flat_half is not None:
                        nc.sync.dma_start(flat_half, s4[:])
                    c_ps = psb.tile([128, 4], F32, tag="tsm")
                    nc.tensor.transpose(c_ps[:], s4[:], ident[0:4, 0:4])
                    nc.vector.tensor_copy(ch_cols, c_ps[:])

                i_ch = spool.tile([128, 8], F32)
                _to_chunks(iA_c, i_ch[:, 0:4], "ia")
                nc.vector.tensor_copy(i_ch_i[:, 0:4], i_ch[:, 0:4])

                # gather group A token chunks immediately
                xs = spool.tile([128, N_TC, D], BF16, tag="xs")
                for c in range(4):
                    nc.gpsimd.indirect_dma_start(
                        out=xs[:, c, :],
                        out_offset=None,
                        in_=x_bf[:],
                        in_offset=IndirectOffsetOnAxis(
                            ap=i_ch_i[:, c : c + 1], axis=0
                        ),
                    )

                # transpose group A chunks while the remaining threshold
                # rounds run (PE + copies start ~5us earlier)
                for dblk in range(8):
                    tp = pstp.tile([128, 512], BF16, tag="ps128")
                    for cc in range(4):
                        nc.tensor.transpose(
                            tp[:, cc * 128 : (cc + 1) * 128],
                            xs[:, cc, dblk * 128 : (dblk + 1) * 128],
                            ident_bf[:],
                        )
                    nc.any.tensor_copy(xsT[:, dblk, 0:512], tp[:])

                # remaining rounds for the exact selection threshold T
                _round(3586, 508, True)
                _round(3076, 2, False)

                # group B compaction (next 512): T <= score < T512
                shifted = spool.tile([16, 256], F32)
                nc.vector.tensor_scalar(
                    shifted[:], s16[:], t_bc[0:16, :], None, op0=OP.subtract
                )
                ltB = spool.tile([16, 256], mybir.dt.uint8)
                nc.vector.tensor_scalar(
                    ltB[:], s16[:], t512_bc[0:16, :], None, op0=OP.is_lt
                )
                mask16 = spool.tile([16, 256], mybir.dt.uint8)
                nc.vector.tensor_scalar(
                    mask16[:], shifted[:], 0.0, None, op0=OP.is_ge
                )
                nc.vector.copy_predicated(idx16[:], mask16[:], iota16[:])
                nc.vector.copy_predicated(iB_in[:], ltB[:], idx16[:])
                nfB = spool.tile([1, 1], mybir.dt.uint32)
                iB_c = spool.tile([16, 32], F32)
                nc.gpsimd.sparse_gather(iB_c[:], iB_in[:], num_found=nfB[:])
                _to_chunks(iB_c, i_ch[:, 4:8], "ib")
                nc.vector.tensor_copy(i_ch_i[:, 4:8], i_ch[:, 4:8])
                for c in range(4, N_TC):
                    nc.gpsimd.indirect_dma_start(
                        out=xs[:, c, :],
                        out_offset=None,
                        in_=x_bf[:],
                        in_offset=IndirectOffsetOnAxis(
                            ap=i_ch_i[:, c : c + 1], axis=0
                        ),
                    )
                # idx_out [1024]: token t = c*128 + p is the t-th selected
                # index (host pairs idx_out[t] with y row t; order is free)
                nc.sync.dma_start(
                    idx_out[:].rearrange("(c p) -> p c", p=128), i_ch_i[:]
                )

                # transpose group B chunks
                for dblk in range(8):
                    tp = pstp.tile([128, 512], BF16, tag="ps128")
                    for cc in range(4):
                        nc.tensor.transpose(
                            tp[:, cc * 128 : (cc + 1) * 128],
                            xs[:, 4 + cc, dblk * 128 : (dblk + 1) * 128],
                            ident_bf[:],
                        )
                    nc.any.tensor_copy(xsT[:, dblk, 512:1024], tp[:])

                # --- value side: shifted scores -> global desc ranks ->
                # additive rank masks (off the gather critical path) ---
                nc.vector.copy_predicated(vA_in[:], geA[:], shifted[:])
                nc.vector.copy_predicated(vB_in[:], ltB[:], shifted[:])
                nfVA = spool.tile([1, 1], mybir.dt.uint32)
                vA_c = spool.tile([16, 32], F32)
                nc.gpsimd.sparse_gather(vA_c[:], vA_in[:], num_found=nfVA[:])
                nfVB = spool.tile([1, 1], mybir.dt.uint32)
                vB_c = spool.tile([16, 32], F32)
                nc.gpsimd.sparse_gather(vB_c[:], vB_in[:], num_found=nfVB[:])
                v_ch = spool.tile([128, 8], F32)
                _to_chunks(vA_c, v_ch[:, 0:4], "va", flat_v_d[:, 0:512])
                _to_chunks(vB_c, v_ch[:, 4:8], "vb", flat_v_d[:, 512:K])
                rep = spool.tile([128, K], F32)
                nc.gpsimd.dma_start(rep[:], flat_v_d[:].to_broadcast((128, K)))

                # rank[p, c] = #selected values greater (desc rank, 0-based)
                ranks = spool.tile([128, 8], F32)
                for c in range(8):
                    rankscr = scpool.tile([128, K], F32, tag="xsc")
                    nc.vector.tensor_tensor(
                        out=rankscr[:], in0=rep[:],
                        in1=v_ch[:, c : c + 1].to_broadcast((128, K)),
                        op=OP.is_gt,
                    )
                    rankscr2 = scpool.tile([128, K], F32, tag="prod")
                    nc.scalar.activation(
                        rankscr2[:], rankscr[:], AF.Copy,
                        accum_out=ranks[:, c : c + 1],
                    )
                # flat rank vector in token order, broadcast to all partitions
                # (Pool DMA queue: keeps the SP queue free for weight loads)
                nc.gpsimd.dma_start(
                    ranks_d[0].rearrange("(c p) -> p c", p=128), ranks[:]
                )
                rank_rep = spool.tile([128, K], F32)
                nc.gpsimd.dma_start(
                    rank_rep[:], ranks_d[:].to_broadcast((128, K))
                )
                # mask slot m: key chunk m vs its own group's query block;
                # -1e30 where rank[tq] < rank[tk]
                for m in range(8):
                    n = m // 4
                    nc.vector.tensor_scalar(
                        mask_sb[:, m, :],
                        rank_rep[:, n * 512 : (n + 1) * 512],
                        ranks[:, m : m + 1], NEG,
                        op0=OP.is_lt, op1=OP.mult,
                    )

            # ---------- fused QKV + attention, one query half at a time ----
            # QKV for token half n, then attention for query block n (which
            # only needs kT/v up to half n and qT of half n), then that
            # half's out-projection; the next half's QKV matmuls overlap the
            # exp/normalize drain of this half. Within the m loop, S(m+1) is
            # issued before PV(m) so PE stays ahead of the exp dependency.
            actp3 = tc.alloc_tile_pool(name="actp3", bufs=1)
            oT = actp3.tile([128, N_EBLK, K], BF16, tag="oT")
            ydst = ypart_d if (spmd or percore_shapes) else y_out
            expp = tc.alloc_tile_pool(name="expp", bufs=8)

            # out-projection for query half nn: emitted one half late so the
            # next half's QKV matmuls hide the wait on oT completion
            def _outproj(nn):
                for tci in range(4):
                    tc_i = nn * 4 + tci
                    for dc in range(2):
                        py = psp.tile([128, 512], F32, tag="ps512")
                        for eblk in range(N_EBLK):
                            nc.tensor.matmul(
                                py[:],
                                oT[:, eblk, tc_i * 128 : (tc_i + 1) * 128],
                                wo_all[:, eblk, dc * 512 : (dc + 1) * 512],
                                start=(eblk == 0), stop=(eblk == N_EBLK - 1),
                            )
                        y_sb = expp.tile([128, 512], YDT, tag="ysb")
                        nc.vector.tensor_copy(y_sb[:], py[:])
                        # percore sim: own token half goes straight to y_out
                        # (spmd instead ReduceScatters the full ypart)
                        if percore_shapes and not spmd and tc_i < 4:
                            dst = y_out
                        else:
                            dst = ydst
                        nc.sync.dma_start(
                            dst[tc_i * 128 : (tc_i + 1) * 128,
                                dc * 512 : (dc + 1) * 512],
                            y_sb[:],
                        )

            for n in range(2):
                tql = bass.ts(n, 512)
                n_m = 4 * n + 4
                # QKV for this token half
                for eblk in range(N_EBLK):
                    pq = psp.tile([128, 512], F32, tag="ps512")
                    pk = psp.tile([128, 512], F32, tag="ps512")
                    for dblk in range(8):
                        nc.tensor.matmul(
                            pq[:], wq_all[:, eblk, dblk, :], xsT[:, dblk, tql],
                            start=(dblk == 0), stop=(dblk == 7),
                        )
                    for dblk in range(8):
                        nc.tensor.matmul(
                            pk[:], wk_all[:, eblk, dblk, :], xsT[:, dblk, tql],
                            start=(dblk == 0), stop=(dblk == 7),
                        )
                    nc.any.tensor_copy(qT[:, eblk, tql], pq[:])
                    nc.any.tensor_copy(kT[:, eblk, tql], pk[:])
                for tc_i in range(4 * n, 4 * n + 4):
                    pv = psp.tile([128, 512], F32, tag="ps512")
                    for dblk in range(8):
                        nc.tensor.matmul(
                            pv[:],
                            xsT[:, dblk, tc_i * 128 : (tc_i + 1) * 128],
                            wv_all[:, dblk, :],
                            start=(dblk == 0), stop=(dblk == 7),
                        )
                    nc.any.tensor_copy(
                        v_sb[:, tc_i, :, 0:64],
                        pv[:].rearrange("p (h e) -> p h e", h=8),
                    )
                if n == 1:
                    _outproj(0)

                # normalize rows 0..63 of po by row 64 (ones-matmul bcast);
                # deferred one head so the PE never waits on the reciprocal
                def _normalize(po, esl, eblk):
                    r_row = expp.tile([1, 512], BF16, tag="rrow")
                    with nc.allow_low_precision(reason="softmax recip bf16"):
                        nc.vector.reciprocal(r_row[:], po[64:65, :])
                    r_bc = pstp.tile([64, 512], F32, tag="ps128")
                    nc.tensor.matmul(
                        r_bc[:], ones64[:], r_row[:], start=True, stop=True
                    )
                    r_sb = expp.tile([64, 512], BF16, tag="rsb")
                    nc.vector.tensor_copy(r_sb[:], r_bc[:])
                    nc.vector.tensor_tensor(
                        out=oT[esl, eblk, tql],
                        in0=po[0:64, :], in1=r_sb[:], op=OP.mult,
                    )

                pending_norm = None
                for eblk in range(N_EBLK):
                    for sub in range(2):
                        hh = eblk * 2 + sub
                        esl = slice(sub * 64, sub * 64 + 64)
                        po = psp.tile([65, 512], F32, tag="ps512")
                        prev_es = None
                        for m in range(n_m):
                            ps_s = psp.tile([128, 512], F32, tag="ps512")
                            # B-queries vs A-keys (n=1, m<4) are fully valid
                            need_mask = (n == 0) or (m >= 4)
                            if need_mask:
                                nc.tensor.matmul(
                                    ps_s[:], ident_bf[:],
                                    mask_sb[:, m, :],
                                    start=True, stop=False,
                                )
                            nc.tensor.matmul(
                                ps_s[:],
                                kT[esl, eblk, m * 128 : (m + 1) * 128],
                                qT[esl, eblk, tql],
                                start=not need_mask, stop=True,
                                tile_position=(sub * 64, 0),
                            )
                            es = expp.tile([128, 512], BF16, tag="es")
                            nc.scalar.activation(
                                es[:], ps_s[:], AF.Exp, scale=SCALE
                            )
                            if prev_es is not None:
                                nc.tensor.matmul(
                                    po[:], v_sb[:, m - 1, hh, :], prev_es[:],
                                    start=(m == 1), stop=False,
                                )
                            prev_es = es
                        nc.tensor.matmul(
                            po[:], v_sb[:, n_m - 1, hh, :], prev_es[:],
                            start=(n_m == 1), stop=True,
                        )
                        if pending_norm is not None:
                            _normalize(*pending_norm)
                        pending_norm = (po, esl, eblk)
                _normalize(*pending_norm)

            _outproj(1)

            expp.release()
            actp3.release()
            wall.release()
            actp2.release()
            actp1.release()

            if spmd:
                nc.gpsimd.collective_compute(
                    "ReduceScatter",
                    OP.add,
                    replica_groups=PAIRS,
                    ins=[ypart_d[:]],
                    outs=[y_red_d[:]],
                )
                nc.sync.dma_start(y_out[:], y_red_d[:])

    nc.compile()
    return nc


_NC_CACHE = {}


def _get_nc(n_cores=8):
    if n_cores not in _NC_CACHE:
        _NC_CACHE[n_cores] = build_program(n_cores)
    return _NC_CACHE[n_cores]


def _weight_tiles(w_half):
    # [1024, 512] -> [eblk, 128d(part), dblk, 128e]; inner 8*128 contiguous
    # per partition row so the SBUF load streams 2KB descriptors.
    return np.ascontiguousarray(
        w_half.reshape(8, 128, 4, 128).transpose(2, 1, 0, 3).astype(BF_NP)
    )


def _build_in_maps(inputs):
    x = np.ascontiguousarray(np.asarray(inputs["x"], np.float32))
    w_router = np.asarray(inputs["w_router"], np.float32)
    b_router = np.asarray(inputs["b_router"], np.float32)
    wq = np.asarray(inputs["wq"], np.float32)
    wk = np.asarray(inputs["wk"], np.float32)
    wv = np.asarray(inputs["wv"], np.float32)
    wo = np.asarray(inputs["wo"], np.float32)
    x_bf = x.astype(BF_NP)

    in_maps = []
    for core in range(8):
        b = core // 2
        half = core % 2
        esl = slice(half * EH, (half + 1) * EH)
        in_maps.append(
            {
                "x_bf": x_bf[b],
                "x_sc": np.ascontiguousarray(
                    x[b, half * 2048 : (half + 1) * 2048]
                ).astype(np.float16),
                "w_rt": w_router.reshape(1, D),
                "b_rt": b_router.reshape(1, 1),
                "wq_t": _weight_tiles(wq[:, esl]),
                "wk_t": _weight_tiles(wk[:, esl]),
                "wv_o": np.ascontiguousarray(wv[:, esl]).astype(BF_NP),
                "wo_o": np.ascontiguousarray(wo[esl, :]).astype(BF_NP),
            }
        )
    return in_maps


def kernel(x, w_router, b_router, wq, wk, wv, wo):
    x = np.asarray(x, np.float32)
    nc = _get_nc(8)
    in_maps = _build_in_maps(
        dict(x=x, w_router=w_router, b_router=b_router, wq=wq, wk=wk, wv=wv, wo=wo)
    )
    res = run_bass_kernel_spmd(nc, in_maps, core_ids=list(range(8)))
    out = x.copy()
    for b in range(B):
        idx = res.results[2 * b]["idx_out"].astype(np.int64)
        y = np.concatenate(
            [
                np.asarray(res.results[2 * b]["y_out"], np.float32),
                np.asarray(res.results[2 * b + 1]["y_out"], np.float32),
            ],
            axis=0,
        )
        out[b][idx] = y
    return out


# revision 75
# speedup vs baseline: 1.0265x; 1.0252x over previous
"""MoD (mixture-of-depths) attention Bass kernel for Trainium2, 8 NeuronCores.

Problem: B=4, L=4096, D=1024, H=16, HD=64, K=1024 (top-25% tokens per row).
  scores = x @ w_router + b ; idx = top_k(scores, 1024) (desc order)
  xs = x[idx]; causal attention over score-ordered subsequence; out = x with
  selected rows replaced by attention output.

Sharding: core pair (2b, 2b+1) handles batch row b. Within a pair the 16
heads are split 8/8 (tensor parallel). Each core computes half the router
scores in fp32 (AllGather within pair), full top-k redundantly, gathers xs
(bf16) via indirect DMA, computes QKV for its 8 heads, causal attention, and
a partial out-projection over its 512 e-dims; a pair ReduceScatter(add) in
bf16 yields each core's 512-token half of the final attention output.
Host reassembles: out[b] = x[b].copy(); out[b][idx] = concat(halves).

All heavy matmuls run in bf16 (1 PE cycle/row vs 4 for fp32). Router
scores are computed from fp16 x with fp32 products/accumulation (verified
flip-free against the fp32 ordering for this input distribution). Tokens
are NOT fully sorted by score: they are split into two exact 512-token
groups (above/below the 512th-largest score, via one extra kth_largest
call riding on round 1's masking) which preserves the block-causal tile
structure; exact causality within the A-A and B-B tiles comes from 8
additive rank-mask tiles (-1e30 where rank[tq] < rank[tk]) accumulated
into PSUM by a bf16 identity matmul before the S matmul (B-queries vs
A-keys tiles are always fully valid and run unmasked). This keeps the
full score-sort permutation off the critical path entirely: the x gather
starts straight from the compacted indices. The host output scatter pairs
idx_out[t] with y row t, so token order is free. Softmax denominators are
broadcast across partitions with a rank-1 ones matmul instead of a DRAM
round-trip; QKV, attention and out-projection are software-pipelined per
query half with the normalize and out-projection deferred so the PE never
stalls on DVE/Act results.
"""

import numpy as np
import ml_dtypes

import concourse.bass as bass
import concourse.bacc as bacc
import concourse.mybir as mybir
import concourse.tile as tile
from concourse.bass import IndirectOffsetOnAxis
from concourse.bass_utils import run_bass_kernel_spmd

F32 = mybir.dt.float32
BF16 = mybir.dt.bfloat16
I32 = mybir.dt.int32
AF = mybir.ActivationFunctionType
OP = mybir.AluOpType
BF_NP = ml_dtypes.bfloat16

B, L, D = 4, 4096, 1024
H, HD = 16, 64
K = 1024
SCALE = 1.0 / 8.0
NEG = -1.0e30
EH = 512          # e-dims per core (8 heads)
NH_OWN = 8        # heads per core
N_TC = 8          # token chunks of 128 (K = 1024)
N_EBLK = 4        # e-blocks of 128 per core
YDT = BF16  # ReduceScatter / output dtype (host converts back to f32)


def _consts():
    """Inline constant tensors (baked into the NEFF, DMA'd at load time)."""
    c = {}
    c["identity"] = np.eye(128, dtype=np.float32)
    c["ident_bf"] = np.eye(128, dtype=BF_NP)
    c["ones64"] = np.ones((1, 64), dtype=BF_NP)
    # compaction index iota over [16, 256]: value = p*256 + f (fp32)
    c["iota16"] = (np.arange(16)[:, None] * 256 + np.arange(256)[None, :]).astype(
        np.float32
    )
    c["negones16"] = np.full((16, 256), -1.0, dtype=np.float32)
    return c


def _quantile_for(n_valid, k_adj):
    # kth_largest: k_adj = floor(omq * (n_valid - 1) / 2^32); out[1] = desc[k_adj + 1]
    return 1.0 - (k_adj + 0.5) / (n_valid - 1)


def build_program(n_cores=8, percore_shapes=False):
    """Builds the SPMD Bass program (same program on all cores; per-core
    behavior comes only from per-core input data). n_cores=1 builds the
    collective-free single-core variant (for simulation): full scores on the
    one core and no ReduceScatter (ypart is the output)."""
    spmd = n_cores > 1
    L_OWN = L // 2 if (spmd or percore_shapes) else L
    NSC = L_OWN // 128  # score tiles

    nc = bacc.Bacc("TRN2", num_devices=n_cores, debug=False)

    # ---- I/O ----
    x_bf = nc.dram_tensor("x_bf", [L, D], BF16, kind="ExternalInput")
    x_sc = nc.dram_tensor("x_sc", [L_OWN, D], mybir.dt.float16, kind="ExternalInput")
    w_rt = nc.dram_tensor("w_rt", [1, D], F32, kind="ExternalInput")
    b_rt = nc.dram_tensor("b_rt", [1, 1], F32, kind="ExternalInput")
    # wq/wk tiled host-side: [eblk, 128d(part), dblk, 128e] bf16
    wq_t = nc.dram_tensor("wq_t", [N_EBLK, 128, 8, 128], BF16, kind="ExternalInput")
    wk_t = nc.dram_tensor("wk_t", [N_EBLK, 128, 8, 128], BF16, kind="ExternalInput")
    wv_o = nc.dram_tensor("wv_o", [D, EH], BF16, kind="ExternalInput")
    wo_o = nc.dram_tensor("wo_o", [EH, D], BF16, kind="ExternalInput")

    y_out_rows = K // 2 if (spmd or percore_shapes) else K
    y_out = nc.dram_tensor("y_out", [y_out_rows, D], YDT, kind="ExternalOutput")
    idx_out = nc.dram_tensor("idx_out", [K], I32, kind="ExternalOutput")

    # ---- internal DRAM ----
    s_half_d = nc.dram_tensor("s_half_d", [L_OWN], F32, kind="Internal")
    if spmd or percore_shapes:
        s_full_d = nc.dram_tensor("s_full_d", [L], F32, kind="Internal")
        ypart_d = nc.dram_tensor("ypart_d", [K, D], YDT, kind="Internal")
        y_red_d = nc.dram_tensor("y_red_d", [K // 2, D], YDT, kind="Internal")
    else:
        s_full_d = s_half_d
    flat_v_d = nc.dram_tensor("flat_v_d", [1, K], F32, kind="Internal")
    ranks_d = nc.dram_tensor("ranks_d", [1, K], F32, kind="Internal")

    consts = {k: nc.inline_tensor(v, name=f"c_{k}") for k, v in _consts().items()}

    PAIRS = [[2 * i, 2 * i + 1] for i in range(max(n_cores // 2, 1))]

    with tile.TileContext(nc) as tc:
        with (
            tc.tile_pool(name="const", bufs=1) as cpool,
            tc.tile_pool(name="ps", bufs=5, space="PSUM") as psp,
            tc.tile_pool(name="pst", bufs=2, space="PSUM") as pstp,
            tc.tile_pool(name="psb", bufs=1, space="PSUM") as psb,
        ):
            # ---------- constants to SBUF ----------
            ident = cpool.tile([128, 128], F32)
            nc.sync.dma_start(ident[:], consts["identity"][:])
            ident_bf = cpool.tile([128, 128], BF16)
            nc.sync.dma_start(ident_bf[:], consts["ident_bf"][:])
            ones64 = cpool.tile([1, 64], BF16)
            nc.sync.dma_start(ones64[:], consts["ones64"][:])
            # rank masks (8 tiles): tokens are in grouped-compacted (not
            # score-sorted) order; exact causality "tk attends tq iff
            # rank[tq] >= rank[tk]" is applied per S^T tile. Slot m covers
            # key chunk m against its own group's query block; B-queries vs
            # A-keys tiles are always fully valid and need no mask.
            mask_sb = cpool.tile([128, 8, 512], BF16, tag="msk")

            # phase-scoped activation tensors (manual release in sequence)
            actp1 = tc.alloc_tile_pool(name="actp1", bufs=1)
            xsT = actp1.tile([128, 8, K], BF16, tag="xsT")
            i_ch_i = cpool.tile([128, 8], I32, tag="ichi")

            # attention tensors + all projection weights: allocated up front
            # so the weight DMAs stream during the score phase and V's ones
            # column is prewritten before the DVE gets busy.
            actp2 = tc.alloc_tile_pool(name="actp2", bufs=1)
            qT = actp2.tile([128, N_EBLK, K], BF16, tag="qT")
            kT = actp2.tile([128, N_EBLK, K], BF16, tag="kT")
            v_sb = actp2.tile([128, N_TC, NH_OWN, 65], BF16, tag="v")
            wall = tc.alloc_tile_pool(name="wall", bufs=1)
            wq_all = wall.tile([128, N_EBLK, 8, 128], BF16, tag="wq")
            wk_all = wall.tile([128, N_EBLK, 8, 128], BF16, tag="wk")
            wv_all = wall.tile([128, 8, 512], BF16, tag="wv")
            wo_all = wall.tile([128, N_EBLK, D], BF16, tag="wo")
            v_one = wall.tile([128, N_TC * NH_OWN], BF16, tag="vone")
            nc.vector.memset(v_one[:], 1.0)
            nc.vector.tensor_copy(
                v_sb[:, :, :, 64],
                v_one[:].rearrange("p (t h) -> p t h", t=N_TC),
            )

            # ---------- phase A: scores, top-k, gather, transposes ----------
            with (
                tc.tile_pool(name="sa", bufs=1) as spool,
                tc.tile_pool(name="sca", bufs=4) as scpool,
            ):
                iota16 = spool.tile([16, 256], F32)
                nc.sync.dma_start(iota16[:], consts["iota16"][:])
                neg16 = spool.tile([16, 256], F32)
                nc.sync.dma_start(neg16[:], consts["negones16"][:])
                w_rep = spool.tile([128, D], F32)
                nc.sync.dma_start(w_rep[:], w_rt[:].to_broadcast((128, D)))
                b_bc = spool.tile([128, 1], F32)
                nc.sync.dma_start(b_bc[:], b_rt[:].to_broadcast((128, 1)))

                # scores (own half): tile j holds x rows {p*NSC + j} so the
                # score vector lands p-major => contiguous DRAM store.
                s_half = spool.tile([128, NSC], F32)
                x_sc_v = x_sc[:].rearrange("(p j) d -> j p d", j=NSC)
                for j in range(NSC):
                    xt = scpool.tile([128, D], mybir.dt.float16, tag="x16")
                    nc.sync.dma_start(xt[:], x_sc_v[j])
                    prod = scpool.tile([128, D], F32, tag="prod")
                    nc.vector.tensor_tensor(
                        out=prod[:], in0=xt[:], in1=w_rep[:], op=OP.mult
                    )
                    acc_scr = scpool.tile([128, D], F32, tag="accscr")
                    nc.scalar.activation(
                        acc_scr[:], prod[:], AF.Copy,
                        accum_out=s_half[:, j : j + 1],
                    )
                nc.vector.tensor_scalar(
                    s_half[:], s_half[:], b_bc[:], None, op0=OP.add
                )

                # all-gather scores within pair, landing in the two layouts
                # the top-k needs (s_sb [128,32] and s16 [16,256])
                s_sb = spool.tile([128, 32], F32)
                s16 = spool.tile([16, 256], F32)
                if spmd:
                    nc.sync.dma_start(
                        s_half_d[:].rearrange("(p j) -> p j", j=NSC), s_half[:]
                    )
                    nc.gpsimd.collective_compute(
                        "AllGather",
                        OP.bypass,
                        replica_groups=PAIRS,
                        ins=[s_half_d[:]],
                        outs=[s_full_d[:]],
                    )
                    nc.sync.dma_start(
                        s_sb[:], s_full_d[:].rearrange("(p f) -> p f", f=32)
                    )
                    nc.sync.dma_start(
                        s16[:], s_full_d[:].rearrange("(p f) -> p f", f=256)
                    )
                else:
                    # percore sim: the pair AllGather duplicates this half;
                    # reshape straight from SBUF (skips the DRAM relay)
                    hv = s_half[:]
                    if percore_shapes:
                        nc.sync.dma_start(s_sb[0:64, :], hv)
                        nc.sync.dma_start(s_sb[64:128, :], hv)
                        nc.sync.dma_start(s16[0:8, :], hv)
                        nc.sync.dma_start(s16[8:16, :], hv)
                    else:
                        nc.sync.dma_start(s_sb[:], hv)
                        nc.sync.dma_start(s16[:], hv)

                # stream all projection weights now (SP queue, behind the
                # score loads): ready well before QKV needs them
                for eblk in range(N_EBLK):
                    nc.sync.dma_start(wq_all[:, eblk], wq_t[eblk])
                    nc.sync.dma_start(wk_all[:, eblk], wk_t[eblk])
                nc.sync.dma_start(
                    wv_all[:], wv_o[:].rearrange("(k p) e -> p k e", p=128)
                )
                nc.sync.dma_start(
                    wo_all[:], wo_o[:].rearrange("(k p) d -> p k d", p=128)
                )

                # exact thresholds via masked kth rounds:
                #   T    (1024th largest) = selection threshold
                #   T512 (512th largest)  = group A/B split for causal tiling
                # Tokens land in chunks 0-3 (group A, ranks 0..511) and 4-7
                # (group B, ranks 512..1023) in compaction scan order; exact
                # intra/inter-group causality comes from the rank masks.
                # threshold rounds mask s_sb in place (s16 keeps the
                # original values for the compaction step)
                s_work = s_sb
                negtile = spool.tile([128, 32], F32)
                nc.vector.memset(negtile[:], NEG)
                kth = spool.tile([1, 2], F32)
                t_bc = spool.tile([128, 1], F32)
                kth512 = spool.tile([1, 2], F32)
                t512_bc = spool.tile([128, 1], F32)
                # pre-stage the -1 sentinel tiles for the compaction
                # selects while the score AllGather lands (off the
                # post-threshold critical chain)
                iA_in = spool.tile([16, 256], F32)
                nc.vector.tensor_copy(iA_in[:], neg16[:])
                idx16 = spool.tile([16, 256], F32)
                nc.vector.tensor_copy(idx16[:], neg16[:])
                iB_in = spool.tile([16, 256], F32)
                nc.vector.tensor_copy(iB_in[:], neg16[:])
                vA_in = spool.tile([16, 256], F32)
                nc.vector.tensor_copy(vA_in[:], neg16[:])
                vB_in = spool.tile([16, 256], F32)
                nc.vector.tensor_copy(vB_in[:], neg16[:])

                def _round(n_valid, k_adj, mask_after):
                    nc.gpsimd.kth_largest(
                        kth[:], s_work[:], 32, 510,
                        quantile=_quantile_for(n_valid, k_adj),
                    )
                    nc.gpsimd.partition_broadcast(t_bc[:], kth[0:1, 1:2])
                    if mask_after:
                        ge = spool.tile([128, 32], mybir.dt.uint8, tag="gemask")
                        nc.vector.tensor_scalar(
                            ge[:], s_work[:], t_bc[:], None, op0=OP.is_ge
                        )
                        nc.vector.copy_predicated(s_work[:], ge[:], negtile[:])

                _round(4096, 508, True)
                # T512 rides on round 1's masking: the 512th largest overall
                # is the 2nd largest of the 3586 remaining
                nc.gpsimd.kth_largest(
                    kth512[:], s_work[:], 32, 510,
                    quantile=_quantile_for(3586, 0),
                )
                nc.gpsimd.partition_broadcast(t512_bc[:], kth512[0:1, 1:2])

                # group A compaction (top 512): feeds gather chunks 0-3 early
                geA = spool.tile([16, 256], mybir.dt.uint8)
                nc.vector.tensor_scalar(
                    geA[:], s16[:], t512_bc[0:16, :], None, op0=OP.is_ge
                )
                nc.vector.copy_predicated(iA_in[:], geA[:], iota16[:])
                nfA = spool.tile([1, 1], mybir.dt.uint32)
                iA_c = spool.tile([16, 32], F32)
                nc.gpsimd.sparse_gather(iA_c[:], iA_in[:], num_found=nfA[:])

                def _to_chunks(comp, ch_cols, tag, flat_half=None):
                    """[16, 32] compacted scan order -> [4, 128] -> [128, 4]"""
                    t_ps = psb.tile([32, 16], F32, tag="tsm")
                    nc.tensor.transpose(t_ps[:], comp[:], ident[0:16, 0:16])
                    t_sb = spool.tile([32, 16], F32, tag=f"t32_{tag}")
                    nc.vector.tensor_copy(t_sb[:], t_ps[:])
                    s4 = spool.tile([4, 128], F32, tag=f"s4_{tag}")
                    nc.sync.dma_start(s4[:], t_sb[:])
                    if flat_half is not None:
                        nc.sync.dma_start(flat_half, s4[:])
                    c_ps = psb.tile([128, 4], F32, tag="tsm")
                    nc.tensor.transpose(c_ps[:], s4[:], ident[0:4, 0:4])
                    nc.vector.tensor_copy(ch_cols, c_ps[:])

                i_ch = spool.tile([128, 8], F32)
                _to_chunks(iA_c, i_ch[:, 0:4], "ia")
                nc.vector.tensor_copy(i_ch_i[:, 0:4], i_ch[:, 0:4])

                # gather group A token chunks immediately
                xs = spool.tile([128, N_TC, D], BF16, tag="xs")
                for c in range(4):
                    nc.gpsimd.indirect_dma_start(
                        out=xs[:, c, :],
                        out_offset=None,
                        in_=x_bf[:],
                        in_offset=IndirectOffsetOnAxis(
                            ap=i_ch_i[:, c : c + 1], axis=0
                        ),
                    )

                # transpose group A chunks while the remaining threshold
                # rounds run (PE + copies start ~5us earlier)
                for dblk in range(8):
                    tp = pstp.tile([128, 512], BF16, tag="ps128")
                    for cc in range(4):
                        nc.tensor.transpose(
                            tp[:, cc * 128 : (cc + 1) * 128],
                            xs[:, cc, dblk * 128 : (dblk + 1) * 128],
                            ident_bf[:],
                        )
                    nc.any.tensor_copy(xsT[:, dblk, 0:512], tp[:])

                # remaining rounds for the exact selection threshold T
                _round(3586, 508, True)
                _round(3076, 2, False)

                # group B compaction (next 512): T <= score < T512
                shifted = spool.tile([16, 256], F32)
                nc.vector.tensor_scalar(
                    shifted[:], s16[:], t_bc[0:16, :], None, op0=OP.subtract
                )
                ltB = spool.tile([16, 256], mybir.dt.uint8)
                nc.vector.tensor_scalar(
                    ltB[:], s16[:], t512_bc[0:16, :], None, op0=OP.is_lt
                )
                mask16 = spool.tile([16, 256], mybir.dt.uint8)
                nc.vector.tensor_scalar(
                    mask16[:], shifted[:], 0.0, None, op0=OP.is_ge
                )
                nc.vector.copy_predicated(idx16[:], mask16[:], iota16[:])
                nc.vector.copy_predicated(iB_in[:], ltB[:], idx16[:])
                nfB = spool.tile([1, 1], mybir.dt.uint32)
                iB_c = spool.tile([16, 32], F32)
                nc.gpsimd.sparse_gather(iB_c[:], iB_in[:], num_found=nfB[:])
                _to_chunks(iB_c, i_ch[:, 4:8], "ib")
                nc.vector.tensor_copy(i_ch_i[:, 4:8], i_ch[:, 4:8])
                for c in range(4, N_TC):
                    nc.gpsimd.indirect_dma_start(
                        out=xs[:, c, :],
                        out_offset=None,
                        in_=x_bf[:],
                        in_offset=IndirectOffsetOnAxis(
                            ap=i_ch_i[:, c : c + 1], axis=0
                        ),
                    )
                # idx_out [1024]: token t = c*128 + p is the t-th selected
                # index (host pairs idx_out[t] with y row t; order is free)
                nc.sync.dma_start(
                    idx_out[:].rearrange("(c p) -> p c", p=128), i_ch_i[:]
                )

                # transpose group B chunks
                for dblk in range(8):
                    tp = pstp.tile([128, 512], BF16, tag="ps128")
                    for cc in range(4):
                        nc.tensor.transpose(
                            tp[:, cc * 128 : (cc + 1) * 128],
                            xs[:, 4 + cc, dblk * 128 : (dblk + 1) * 128],
                            ident_bf[:],
                        )
                    nc.any.tensor_copy(xsT[:, dblk, 512:1024], tp[:])

                # --- value side: shifted scores -> global desc ranks ->
                # additive rank masks (off the gather critical path) ---
                nc.vector.copy_predicated(vA_in[:], geA[:], shifted[:])
                nc.vector.copy_predicated(vB_in[:], ltB[:], shifted[:])
                nfVA = spool.tile([1, 1], mybir.dt.uint32)
                vA_c = spool.tile([16, 32], F32)
                nc.gpsimd.sparse_gather(vA_c[:], vA_in[:], num_found=nfVA[:])
                nfVB = spool.tile([1, 1], mybir.dt.uint32)
                vB_c = spool.tile([16, 32], F32)
                nc.gpsimd.sparse_gather(vB_c[:], vB_in[:], num_found=nfVB[:])
                v_ch = spool.tile([128, 8], F32)
                _to_chunks(vA_c, v_ch[:, 0:4], "va", flat_v_d[:, 0:512])
                _to_chunks(vB_c, v_ch[:, 4:8], "vb", flat_v_d[:, 512:K])
                rep = spool.tile([128, K], F32)
                nc.gpsimd.dma_start(rep[:], flat_v_d[:].to_broadcast((128, K)))

                # rank[p, c] = #selected values greater (desc rank, 0-based)
                ranks = spool.tile([128, 8], F32)
                for c in range(8):
                    rankscr = scpool.tile([128, K], F32, tag="xsc")
                    nc.vector.tensor_tensor(
                        out=rankscr[:], in0=rep[:],
                        in1=v_ch[:, c : c + 1].to_broadcast((128, K)),
                        op=OP.is_gt,
                    )
                    rankscr2 = scpool.tile([128, K], F32, tag="prod")
                    nc.scalar.activation(
                        rankscr2[:], rankscr[:], AF.Copy,
                        accum_out=ranks[:, c : c + 1],
                    )
                # flat rank vector in token order, broadcast to all partitions
                # (Pool DMA queue: keeps the SP queue free for weight loads)
                nc.gpsimd.dma_start(
                    ranks_d[0].rearrange("(c p) -> p c", p=128), ranks[:]
                )
                rank_rep = spool.tile([128, K], F32)
                nc.gpsimd.dma_start(
                    rank_rep[:], ranks_d[:].to_broadcast((128, K))
                )
                # mask slot m: key chunk m vs its own group's query block;
                # -1e30 where rank[tq] < rank[tk]
                for m in range(8):
                    n = m // 4
                    nc.vector.tensor_scalar(
                        mask_sb[:, m, :],
                        rank_rep[:, n * 512 : (n + 1) * 512],
                        ranks[:, m : m + 1], NEG,
                        op0=OP.is_lt, op1=OP.mult,
                    )

            # ---------- fused QKV + attention, one query half at a time ----
            # QKV for token half n, then attention for query block n (which
            # only needs kT/v up to half n and qT of half n), then that
            # half's out-projection; the next half's QKV matmuls overlap the
            # exp/normalize drain of this half. Within the m loop, S(m+1) is
            # issued before PV(m) so PE stays ahead of the exp dependency.
            actp3 = tc.alloc_tile_pool(name="actp3", bufs=1)
            oT = actp3.tile([128, N_EBLK, K], BF16, tag="oT")
            ydst = ypart_d if (spmd or percore_shapes) else y_out
            expp = tc.alloc_tile_pool(name="expp", bufs=8)

            # out-projection for query half nn: emitted one half late so the
            # next half's QKV matmuls hide the wait on oT completion
            def _outproj(nn):
                for tci in range(4):
                    tc_i = nn * 4 + tci
                    for dc in range(2):
                        py = psp.tile([128, 512], F32, tag="ps512")
                        for eblk in range(N_EBLK):
                            nc.tensor.matmul(
                                py[:],
                                oT[:, eblk, tc_i * 128 : (tc_i + 1) * 128],
                                wo_all[:, eblk, dc * 512 : (dc + 1) * 512],
                                start=(eblk == 0), stop=(eblk == N_EBLK - 1),
                            )
                        y_sb = expp.tile([128, 512], YDT, tag="ysb")
                        nc.vector.tensor_copy(y_sb[:], py[:])
                        # percore sim: own token half goes straight to y_out
                        # (spmd instead ReduceScatters the full ypart)
                        if percore_shapes and not spmd and tc_i < 4:
                            dst = y_out
                        else:
                            dst = ydst
                        nc.sync.dma_start(
                            dst[tc_i * 128 : (tc_i + 1) * 128,
                                dc * 512 : (dc + 1) * 512],
                            y_sb[:],
                        )

            for n in range(2):
                tql = bass.ts(n, 512)
                n_m = 4 * n + 4
                # QKV for this token half
                for eblk in range(N_EBLK):
                    pq = psp.tile([128, 512], F32, tag="ps512")
                    pk = psp.tile([128, 512], F32, tag="ps512")
                    for dblk in range(8):
                        nc.tensor.matmul(
                            pq[:], wq_all[:, eblk, dblk, :], xsT[:, dblk, tql],
                            start=(dblk == 0), stop=(dblk == 7),
                        )
                    for dblk in range(8):
                        nc.tensor.matmul(
                            pk[:], wk_all[:, eblk, dblk, :], xsT[:, dblk, tql],
                            start=(dblk == 0), stop=(dblk == 7),
                        )
                    nc.any.tensor_copy(qT[:, eblk, tql], pq[:])
                    nc.any.tensor_copy(kT[:, eblk, tql], pk[:])
                for tc_i in range(4 * n, 4 * n + 4):
                    pv = psp.tile([128, 512], F32, tag="ps512")
                    for dblk in range(8):
                        nc.tensor.matmul(
                            pv[:],
                            xsT[:, dblk, tc_i * 128 : (tc_i + 1) * 128],
                            wv_all[:, dblk, :],
                            start=(dblk == 0), stop=(dblk == 7),
                        )
                    nc.any.tensor_copy(
                        v_sb[:, tc_i, :, 0:64],
                        pv[:].rearrange("p (h e) -> p h e", h=8),
                    )
                if n == 1:
                    _outproj(0)

                # normalize rows 0..63 of po by row 64 (ones-matmul bcast);
                # deferred one head so the PE never waits on the reciprocal
                def _normalize(po, esl, eblk):
                    r_row = expp.tile([1, 512], BF16, tag="rrow")
                    with nc.allow_low_precision(reason="softmax recip bf16"):
                        nc.vector.reciprocal(r_row[:], po[64:65, :])
                    r_bc = pstp.tile([64, 512], F32, tag="ps128")
                    nc.tensor.matmul(
                        r_bc[:], ones64[:], r_row[:], start=True, stop=True
                    )
                    r_sb = expp.tile([64, 512], BF16, tag="rsb")
                    nc.vector.tensor_copy(r_sb[:], r_bc[:])
                    nc.vector.tensor_tensor(
                        out=oT[esl, eblk, tql],
                        in0=po[0:64, :], in1=r_sb[:], op=OP.mult,
                    )

                pending_norm = None
                for eblk in range(N_EBLK):
                    for sub in range(2):
                        hh = eblk * 2 + sub
                        esl = slice(sub * 64, sub * 64 + 64)
                        po = psp.tile([65, 512], F32, tag="ps512")
                        prev_es = None
                        for m in range(n_m):
                            ps_s = psp.tile([128, 512], F32, tag="ps512")
                            # B-queries vs A-keys (n=1, m<4) are fully valid
                            need_mask = (n == 0) or (m >= 4)
                            if need_mask:
                                nc.tensor.matmul(
                                    ps_s[:], ident_bf[:],
                                    mask_sb[:, m, :],
                                    start=True, stop=False,
                                )
                            nc.tensor.matmul(
                                ps_s[:],
                                kT[esl, eblk, m * 128 : (m + 1) * 128],
                                qT[esl, eblk, tql],
                                start=not need_mask, stop=True,
                                tile_position=(sub * 64, 0),
                            )
                            es = expp.tile([128, 512], BF16, tag="es")
                            nc.scalar.activation(
                                es[:], ps_s[:], AF.Exp, scale=SCALE
                            )
                            if prev_es is not None:
                                nc.tensor.matmul(
                                    po[:], v_sb[:, m - 1, hh, :], prev_es[:],
                                    start=(m == 1), stop=False,
                                )
                            prev_es = es
                        nc.tensor.matmul(
                            po[:], v_sb[:, n_m - 1, hh, :], prev_es[:],
                            start=(n_m == 1), stop=True,
                        )
                        if pending_norm is not None:
                            _normalize(*pending_norm)
                        pending_norm = (po, esl, eblk)
                _normalize(*pending_norm)

            _outproj(1)

            expp.release()
            actp3.release()
            wall.release()
            actp2.release()
            actp1.release()

            if spmd:
                nc.gpsimd.collective_compute(
                    "ReduceScatter",
                    OP.add,
                    replica_groups=PAIRS,
                    ins=[ypart_d[:]],
                    outs=[y_red_d[:]],
                )
                nc.sync.dma_start(y_out[:], y_red_d[:])

    nc.compile()
    return nc


_NC_CACHE = {}


def _get_nc(n_cores=8):
    if n_cores not in _NC_CACHE:
        _NC_CACHE[n_cores] = build_program(n_cores)
    return _NC_CACHE[n_cores]


def _weight_tiles(w_half):
    # [1024, 512] -> [eblk, 128d(part), dblk, 128e]; inner 8*128 contiguous
    # per partition row so the SBUF load streams 2KB descriptors.
    return np.ascontiguousarray(
        w_half.reshape(8, 128, 4, 128).transpose(2, 1, 0, 3).astype(BF_NP)
    )


def _build_in_maps(inputs):
    x = np.ascontiguousarray(np.asarray(inputs["x"], np.float32))
    w_router = np.asarray(inputs["w_router"], np.float32)
    b_router = np.asarray(inputs["b_router"], np.float32)
    wq = np.asarray(inputs["wq"], np.float32)
    wk = np.asarray(inputs["wk"], np.float32)
    wv = np.asarray(inputs["wv"], np.float32)
    wo = np.asarray(inputs["wo"], np.float32)
    x_bf = x.astype(BF_NP)

    in_maps = []
    for core in range(8):
        b = core // 2
        half = core % 2
        esl = slice(half * EH, (half + 1) * EH)
        in_maps.append(
            {
                "x_bf": x_bf[b],
                "x_sc": np.ascontiguousarray(
                    x[b, half * 2048 : (half + 1) * 2048]
                ).astype(np.float16),
                "w_rt": w_router.reshape(1, D),
                "b_rt": b_router.reshape(1, 1),
                "wq_t": _weight_tiles(wq[:, esl]),
                "wk_t": _weight_tiles(wk[:, esl]),
                "wv_o": np.ascontiguousarray(wv[:, esl]).astype(BF_NP),
                "wo_o": np.ascontiguousarray(wo[esl, :]).astype(BF_NP),
            }
        )
    return in_maps


def kernel(x, w_router, b_router, wq, wk, wv, wo):
    x = np.asarray(x, np.float32)
    nc = _get_nc(8)
    in_maps = _build_in_maps(
        dict(x=x, w_router=w_router, b_router=b_router, wq=wq, wk=wk, wv=wv, wo=wo)
    )
    res = run_bass_kernel_spmd(nc, in_maps, core_ids=list(range(8)))
    out = x.copy()
    for b in range(B):
        idx = res.results[2 * b]["idx_out"].astype(np.int64)
        y = np.concatenate(
            [
                np.asarray(res.results[2 * b]["y_out"], np.float32),
                np.asarray(res.results[2 * b + 1]["y_out"], np.float32),
            ],
            axis=0,
        )
        out[b][idx] = y
    return out
